# revision 1
# baseline (speedup 1.0000x reference)
"""Trainium2 Bass kernel for nn_CDFL1HistogramLoss (CDF-L1 histogram loss).

Math (derived from the reference):
  1. jax.image.resize(bilinear, 512->256, antialiased) is a separable 4-tap
     filter: interior out[i] = (x[2i-1] + 3x[2i] + 3x[2i+1] + x[2i+2])/8,
     edges [3,3,1]/7.  Both passes run on the PE as matmuls against the
     constant 512x256 band matrix MH.  The vertical pass uses the raw image
     chunks as the stationary operand so its output lands transposed
     (w on partitions), which lets the horizontal pass contract over w with
     no transposes; the full MH columns absorb the edge renormalization.
  2. The soft histogram telescopes: with u = 256*x and c = SIGMA/256,
     cumsum(hist)[k] = T(0) - T(k+1) where T(t) = sum_x sigmoid(c*(u - t)).
  3. sigmoid saturates ~8 bins away, so each pixel only contributes
     non-trivially to a window around its own value.  Anchoring windows at
     16-bin coarse buckets (h = round(u/16), w = u - 16h in [-8,8]) gives
     A[m, j] = sum_{x in bucket m} sigmoid(c*(w - t_j)) for t_j in
     [T_LO, T_HI], plus moment columns sum (w/16)^p, computed as
     PSUM-accumulated 128-pixel matmuls of (coarse one-hot) x (sigmoid
     columns + moments + ones).  T (and the CDF numerators C[k]) are a
     fixed linear map R2 of A, precomputed on host (saturated tails use a
     least-squares fit of the sigmoid in powers of w/16 over w~U[-8,8],
     contracted against the device moments).
  4. Per-channel loss = mean_k |Cp[k]/Cp[255] - Ct[k]/Ct[255]| computed
     host-side in float64 from the A matrices of the 8 cores.

Quantization trick: v + 128 cast to bf16 rounds to the nearest integer
(ulp = 1 on [128, 256)), giving the bucket h + 128 without an i32 round
trip; w/16 = (v + 128) - bf16(v + 128).  NOTE: the f32->i32 DVE cast
rounds to nearest (it does not truncate), so the i32 formulation the
baseline used actually computed h = ceil(v), putting w in (-16, 0] and
silently breaking the R2 tail assumptions (that was its ~1.3e-2 error).

Engine assignment (per channel):
  PE:    both resize passes (bf16 operands, f32 PSUM accumulation),
         scatter (fp8 one-hot stationary x bf16 sigs moving, 4 PE column
         strips), transpose of the A accumulator.
  ACT:   the N_T sigmoid columns, batched two channels per instruction
         ([128, 1024] contiguous writes).
  DVE:   quantization, one-hot, moment columns, PSUM->SBUF copies.

The channel loop is software-pipelined at channel-pair granularity (the
resize/quantize stage of pair p+1 is emitted before the sigmoid/scatter
stage of pair p) so the ACT and PE engines stay fed.

Sharding: data-parallel over batch N: core i handles batches [2i, 2i+1] of
both pred and target (12 channel-histograms, 6 pred/target pairs per core).
"""
import os
import numpy as np

import concourse.bass as bass
import concourse.bacc as bacc
import concourse.mybir as mybir
from concourse import tile
from concourse.bass_utils import run_bass_kernel_spmd

F32 = mybir.dt.float32
BF16 = mybir.dt.bfloat16
FP8 = mybir.dt.float8e4
I32 = mybir.dt.int32
ALU = mybir.AluOpType
ACT = mybir.ActivationFunctionType
DR = mybir.MatmulPerfMode.DoubleRow

N_CORES = 8
BINS = 256
SIGMA = 300.0
C = SIGMA / BINS          # 1.171875
N_M = 17                  # coarse buckets h = round(u/16) in [0, 16]
T_LO, T_HI = -8, 8        # sigmoid column offsets (window +-8 fine bins)
N_T = T_HI - T_LO + 1     # 17 sigmoid columns
N_MOM = 3                 # w, w^2, w^3 moment columns for the tail fits
NT = N_T + N_MOM + 1      # + moments + ones column
NPIX = 65536              # pixels per channel after resize


def make_mh() -> np.ndarray:
    """[512, 256] vertical resize matrix (jax bilinear antialiased 2x down)."""
    M = np.zeros((512, 256), dtype=np.float64)
    for i in range(256):
        if i == 0:
            M[0, 0], M[1, 0], M[2, 0] = 3 / 7, 3 / 7, 1 / 7
        elif i == 255:
            M[509, 255], M[510, 255], M[511, 255] = 1 / 7, 3 / 7, 3 / 7
        else:
            M[2 * i - 1, i] = 1 / 8
            M[2 * i, i] = 3 / 8
            M[2 * i + 1, i] = 3 / 8
            M[2 * i + 2, i] = 1 / 8
    return M.astype(np.float32)


def make_r2() -> np.ndarray:
    """R2[m, j, k]: maps A[m, j] -> C[k] = (T(0) - T(k+1))/NPIX, k = 0..255.

    A columns j = 0..N_T-1 are sigmoid(c*(w - t_j)), t_j = T_LO + j;
    columns N_T..N_T+N_MOM-1 are bucket moments sum((w/16)^p), p=1..N_MOM;
    column NT-1 is the bucket count.  For T(tp), bucket m contributes the
    exact column when trel = tp - 16m is in window, else the least-squares
    fit of sigmoid(c*(w - trel)) in powers of w/16 over w ~ U[-8, 8]
    (smooth saturation tail).
    """
    def sig(z):
        return 1.0 / (1.0 + np.exp(-z))

    wq = np.linspace(-8.0, 8.0, 8193)
    wvq = wq / 16.0
    Vq = np.stack([wvq ** p for p in range(N_MOM + 1)], axis=1)
    Rfull = np.zeros((N_M, NT, 257))
    for m in range(N_M):
        for tp in range(257):
            trel = tp - 16 * m
            if T_LO <= trel <= T_HI:
                Rfull[m, trel - T_LO, tp] = 1.0
            else:
                y = sig(C * (wq - trel))
                coef, *_ = np.linalg.lstsq(Vq, y, rcond=None)
                Rfull[m, NT - 1, tp] = coef[0]
                for p in range(1, N_MOM + 1):
                    Rfull[m, N_T + p - 1, tp] = coef[p]
    R2 = (Rfull[:, :, 0:1] - Rfull[:, :, 1:257]) / float(NPIX)
    return R2.astype(np.float32)  # [N_M, NT, 256]


def _nonzero_blocks(MH):
    """Which (half, q) 128x128 blocks of MH are nonzero."""
    blocks = {}
    for half in range(2):
        qs = []
        for q in range(4):
            blk = MH[128 * q:128 * (q + 1), 128 * half:128 * (half + 1)]
            if np.any(blk != 0):
                qs.append(q)
        blocks[half] = qs
    return blocks


def build(n_pairs: int = 6, bf16_resize: bool = True, split_scatter: bool = False):
    """Build the per-core Bass program. Channels: n_pairs pred + n_pairs target."""
    MH = make_mh()
    mh_blocks = _nonzero_blocks(MH)
    n_ch = 2 * n_pairs

    nc = bacc.Bacc("TRN2", target_bir_lowering=False, debug=False, num_devices=N_CORES)
    pred = nc.dram_tensor("pred", [2, 3, 512, 512], F32, kind="ExternalInput").ap()
    target = nc.dram_tensor("target", [2, 3, 512, 512], F32, kind="ExternalInput").ap()
    mh = nc.dram_tensor("mh", [512, 256], F32, kind="ExternalInput").ap()
    out = nc.dram_tensor("out", [NT, n_ch * N_M], F32, kind="ExternalOutput").ap()

    with tile.TileContext(nc) as tc:
        from contextlib import ExitStack
        nv = nc.vector
        ns = nc.scalar
        ng = nc.gpsimd
        ctx = ExitStack()
        cpool = ctx.enter_context(tc.tile_pool(name="consts", bufs=1))

        # ---- constants in SBUF ----
        mh_sb = cpool.tile(shape=[128, 4, 256], dtype=F32, name="mh_sb")
        nc.sync.dma_start(mh_sb, mh.rearrange("(q p) w -> p q w", p=128))
        id_sb = cpool.tile(shape=[128, 128], dtype=F32, name="id_sb")
        idi = cpool.tile(shape=[128, 128], dtype=I32, name="idi")
        nc.gpsimd.iota(idi, pattern=[[1, 128]], base=0, channel_multiplier=-1)
        nv.tensor_scalar(id_sb, idi, 0.0, None, ALU.is_equal)
        iota_i = cpool.tile(shape=[128, 32], dtype=I32, name="iota_i")
        nc.gpsimd.iota(iota_i, pattern=[[1, 32]], base=0, channel_multiplier=0)
        # bias[j] = -C * t_j = -C*(j + T_LO)
        bias_sb = cpool.tile(shape=[128, N_T], dtype=F32, name="bias_sb")
        nv.tensor_scalar(bias_sb, iota_i[:, 0:N_T], -C, -C * T_LO, ALU.mult, ALU.add)
        zero_sb = cpool.tile(shape=[128, 1], dtype=F32, name="zero_sb")
        nc.gpsimd.memset(zero_sb, 0.0)
        # 16*MH for the horizontal resize: output v = 16*resized = u/16 in [0,16]
        mh2_sb = cpool.tile(shape=[128, 4, 256], dtype=F32, name="mh2_sb")
        nv.tensor_scalar(mh2_sb, mh_sb, 16.0, None, ALU.mult)
        if bf16_resize:
            mhb_sb = cpool.tile(shape=[128, 4, 256], dtype=BF16, name="mhb_sb")
            nv.tensor_copy(mhb_sb, mh_sb)
            mh2b_sb = cpool.tile(shape=[128, 4, 256], dtype=BF16, name="mh2b_sb")
            nv.tensor_copy(mh2b_sb, mh2_sb)

        # double-buffered sigmoid-column tensors; the ones column (index N_T)
        # is constant so it is written once per buffer here, not per channel
        # double-buffered paired sigmoid tensors: [128, NT, pair-slot, 512]
        sigs_ab = []
        for s in range(2):
            sg = cpool.tile(shape=[128, NT, 2, 512], dtype=BF16, name=f"sigs{s}")
            nc.gpsimd.memset(sg[:, NT - 1, :, :], 1.0)
            sigs_ab.append(sg)

        # A^T stack: partition = sigmoid-column j, free = (channel, coarse bucket m)
        a_all = cpool.tile(shape=[NT, n_ch, N_M], dtype=F32, name="a_all")

        # ---- per-channel pipeline ----
        ch_ctx = ExitStack()
        io_pool = ch_ctx.enter_context(tc.tile_pool(name="io", bufs=3))
        wk_pool = ch_ctx.enter_context(tc.tile_pool(name="wk", bufs=3))
        ab_pool = ch_ctx.enter_context(tc.tile_pool(name="ab", bufs=3))
        hot_pool = ch_ctx.enter_context(tc.tile_pool(name="hot", bufs=4))
        hp_pool = ch_ctx.enter_context(tc.tile_pool(name="hp", bufs=2, space="PSUM"))
        up2_pool = ch_ctx.enter_context(tc.tile_pool(name="up2", bufs=1, space="PSUM"))
        at_pool = ch_ctx.enter_context(tc.tile_pool(name="at", bufs=2, space="PSUM"))

        chans = []
        for pi in range(n_pairs):
            chans.append(("p", pi))
        for pi in range(n_pairs):
            chans.append(("t", pi))

        # --- stage A: dma + resize + quantize; fills slot ci&1 of the pair's
        # (wbf2, hbf2) ---
        def stage_a(ci, wbf2, hbf2):
            grp, pi = chans[ci]
            b, cch = divmod(pi, 3)
            src = (pred if grp == "p" else target)[b, cch]  # [512, 512] dram
            raw = io_pool.tile(shape=[128, 4, 512], dtype=F32, name="raw")
            nc.sync.dma_start(raw, src.rearrange("(q p) w -> p q w", p=128))

            # vertical resize (PE), output transposed directly by using the
            # raw image chunks as the stationary operand:
            # hsT[w, (ih, i)] = sum_r raw[r, w] * MH[r, ih*128+i]
            # (same per-element f32 accumulation order as the row-major form)
            if bf16_resize:
                rawv = wk_pool.tile(shape=[128, 4, 512], dtype=BF16, name="rawb")
                nv.tensor_copy(rawv, raw)
                mhv = mhb_sb
            else:
                rawv, mhv = raw, mh_sb
            hpt = hp_pool.tile(shape=[128, 4, 2, 128], dtype=F32, space="PSUM", name="hpt")
            for wc in range(4):
                for ih in range(2):
                    qs = mh_blocks[ih]
                    for qi, q in enumerate(qs):
                        nc.tensor.matmul(
                            hpt[:, wc, ih, :], rawv[:, q, 128 * wc:128 * (wc + 1)],
                            mhv[:, q, 128 * ih:128 * (ih + 1)],
                            start=(qi == 0), stop=(qi == len(qs) - 1),
                        )
            hst = wk_pool.tile(shape=[128, 4, 2, 128],
                               dtype=BF16 if bf16_resize else F32, name="hst")
            nv.tensor_copy(hst, hpt)
            mh2v = mh2b_sb if bf16_resize else mh2_sb

            # horizontal resize (PE): v = 16 * resized = u/16 in [0, 16]
            # up2[oc, (ih, i)] = sum_wc mh2[wc-block, oh-half]^T @ hsT[:, wc, :, :]
            # full MH columns include the edge renormalization; no edge ops.
            v128 = wk_pool.tile(shape=[128, 2, 2, 128], dtype=F32, name="v128")
            for oh in range(2):
                up2_ps = up2_pool.tile(shape=[128, 2, 128], dtype=F32, space="PSUM", name="up2_ps")
                qs = mh_blocks[oh]
                for qi, q in enumerate(qs):
                    nc.tensor.matmul(
                        up2_ps, mh2v[:, q, 128 * oh:128 * (oh + 1)], hst[:, q, :, :],
                        start=(qi == 0), stop=(qi == len(qs) - 1),
                    )
                # v + 128: bf16 cast of this rounds to nearest integer
                # (ulp = 1 on [128, 256)), giving the bucket h + 128
                nv.tensor_scalar(v128[:, oh], up2_ps, 1.0, 128.0, ALU.mult, ALU.add)

            v128f = v128.rearrange("p a h i -> p (a h i)")
            hbf = hbf2[:, ci & 1, :]
            nv.tensor_copy(hbf, v128f)  # h + 128, exact integers in bf16
            wbf = wbf2[:, ci & 1, :]
            nv.tensor_tensor(wbf, v128f, hbf, ALU.subtract)

            # moment columns and the one-hot depend only on (wbf, hbf), so
            # they are emitted here in stage A: the scatter of this pair then
            # starts as soon as its last ACT sigmoid lands, with no DVE tail
            # on the critical path.
            sigs = sigs_ab[(ci // 2) % 2][:, :, ci & 1, :]
            nv.tensor_copy(sigs[:, N_T, :], wbf)
            nv.tensor_tensor(sigs[:, N_T + 1, :], wbf, wbf, ALU.mult)
            nv.tensor_tensor(sigs[:, N_T + 2, :], sigs[:, N_T + 1, :], wbf, ALU.mult)
            hi = hot_pool.tile(shape=[128, N_M, 512], dtype=FP8, name="hi")
            for m in range(N_M):
                nv.tensor_scalar(hi[:, m, :], hbf, float(m + 128), None, ALU.is_equal)
            return hi

        # --- stage B (per pair): paired sigmoid columns, then per-channel
        # scatter + extraction ---
        def stage_b_pair(p, wbf2, hbf2, hi_pair):
            # sigmoid columns for both channels in one [128, 1024] pass per j
            sigs2 = sigs_ab[p % 2]
            wflat = wbf2.rearrange("p s w -> p (s w)")
            for j in range(N_T):
                ns.activation(sigs2[:, j, :, :].rearrange("p s w -> p (s w)"),
                              wflat, ACT.Sigmoid,
                              bias=bias_sb[:, j:j + 1], scale=16.0 * C)
            for slot in range(2):
                stage_b(2 * p + slot, sigs2, hi_pair[slot])

        def stage_b(ci, sigs2, hi):
            slot = ci & 1
            sigs = sigs2[:, :, slot, :]

            # scatter: A[m, j] += onehot^T @ sigs over all 512 pixel-columns.
            # Each pixel-column is split into two 64-row contraction halves in
            # separate PE row-groups; with 4 column strips that is 8
            # concurrent 64x32 PE tiles, doubling the weight-load stream
            # parallelism (the scatter is LDWEIGHTS-bandwidth-bound).
            G = 4
            a_ps4 = at_pool.tile(shape=[128, 512], dtype=F32, space="PSUM", name="a_ps4")
            for f in range(512):
                g = f % G
                if split_scatter:
                    for r in range(2):
                        nc.tensor.matmul(
                            a_ps4[32 * g:32 * g + N_M, 0:NT],
                            hi[64 * r:64 * r + 64, :, f], sigs[64 * r:64 * r + 64, :, f],
                            start=(f < G and r == 0), stop=(f >= 512 - G and r == 1),
                            tile_position=(64 * r, 32 * g), skip_group_check=True)
                else:
                    nc.tensor.matmul(
                        a_ps4[32 * g:32 * g + N_M, 0:NT], hi[:, :, f], sigs[:, :, f],
                        start=(f < G), stop=(f >= 512 - G),
                        tile_position=(0, 32 * g), skip_group_check=True)
            # single full-height copy: rows outside the 4 N_M strips were
            # zeroed by start=True and are never read downstream
            aps_sb = wk_pool.tile(shape=[128, NT], dtype=F32, name="aps_sb")
            nv.tensor_copy(aps_sb, a_ps4[:, 0:NT])
            at_t = up2_pool.tile(shape=[NT, 128], dtype=F32, space="PSUM", name="at_t")
            nc.tensor.transpose(at_t, aps_sb, id_sb)
            att_sb = wk_pool.tile(shape=[NT, 128], dtype=F32, name="att_sb")
            nv.tensor_copy(att_sb, at_t)
            nv.tensor_tensor(a_all[:, ci, :], att_sb[:, 0:N_M], att_sb[:, 32:32 + N_M], ALU.add)
            nv.tensor_tensor(a_all[:, ci, :], a_all[:, ci, :], att_sb[:, 64:64 + N_M], ALU.add)
            nv.tensor_tensor(a_all[:, ci, :], a_all[:, ci, :], att_sb[:, 96:96 + N_M], ALU.add)

        # software pipeline at pair granularity: the A stages of pair p+1 are
        # emitted before the B stage of pair p so every engine stays fed
        n_pair = n_ch // 2
        pending = {}
        for p in range(n_pair + 1):
            if p < n_pair:
                wbf2 = ab_pool.tile(shape=[128, 2, 512], dtype=BF16, name="wbf2")
                hbf2 = ab_pool.tile(shape=[128, 2, 512], dtype=BF16, name="hbf2")
                hi0 = stage_a(2 * p, wbf2, hbf2)
                hi1 = stage_a(2 * p + 1, wbf2, hbf2)
                pending[p] = (wbf2, hbf2, (hi0, hi1))
            if p >= 1:
                stage_b_pair(p - 1, *pending.pop(p - 1))

        ch_ctx.close()
        nc.sync.dma_start(out, a_all.rearrange("j c m -> j (c m)"))
        ctx.close()

    nc.compile()
    return nc


_CACHE: dict = {}
LAST_RESULT = None


def _get_nc(n_pairs=6):
    key = n_pairs
    if key not in _CACHE:
        _CACHE[key] = build(n_pairs)
    return _CACHE[key]


def kernel(pred: np.ndarray, target: np.ndarray) -> np.ndarray:
    global LAST_RESULT
    pred = np.ascontiguousarray(pred, dtype=np.float32)
    target = np.ascontiguousarray(target, dtype=np.float32)
    assert pred.shape == (16, 3, 512, 512) and target.shape == (16, 3, 512, 512)

    nc = _get_nc(6)
    mh_buf = make_mh()
    in_maps = []
    for i in range(N_CORES):
        in_maps.append({
            "pred": pred[2 * i:2 * i + 2],
            "target": target[2 * i:2 * i + 2],
            "mh": mh_buf,
        })
    trace = os.environ.get("KERNEL_TRACE", "0") == "1"
    res = run_bass_kernel_spmd(nc, in_maps, core_ids=list(range(N_CORES)), trace=trace)
    LAST_RESULT = res
    # host-side unbinning: C[k] per channel from A via R2 (float64), then loss
    R2 = make_r2().astype(np.float64).reshape(N_M * NT, 256)  # [(m, j), k]
    losses = []
    for i in range(N_CORES):
        a = res.results[i]["out"].astype(np.float64).reshape(NT, 12, N_M)
        a = a.transpose(1, 2, 0).reshape(12, N_M * NT)  # [ch, (m, j)]
        Cn = a @ R2  # [12, 256]
        for p in range(6):
            Cp, Ct = Cn[p], Cn[p + 6]
            losses.append(np.mean(np.abs(Cp / Cp[-1] - Ct / Ct[-1])))
    return np.float32(np.mean(losses))

